# revision 17
# baseline (speedup 1.0000x reference)
"""Causal multi-head attention (B=1, S=4096, H=16, D=128) on 8 TRN2 NeuronCores.

Sharding: pure head-parallel SPMD - 16 heads / 8 cores = 2 heads per core.
Each core receives its heads' Q^T, K^T (pre-transposed to [D, S] on host) and
V ([S, D]), and computes full causal attention for those heads. No collectives.

Per-core kernel layout ("layout A" - scores computed transposed):
  S^T[t, q] chunk = matmul(lhsT=K^T[:, tchunk], rhs=Q^T[:, qblock])  (PE)
  P^T = exp(S^T * 1/sqrt(D))  PSUM -> SBUF fp16                      (ACT)
  causal mask fix-up on diagonal chunks (tri-mask mul + memset)      (DVE/GPSIMD)
  out^T[d, q] += matmul(lhsT=V[tchunk], rhs=P^T chunk)  accum PSUM   (PE)
  l[q] = ones^T @ (pairwise-tree-sum of P^T chunks)                  (DVE tree + PE)
  out = out^T * 1/l                                                  (DVE)
Host reassembles [B, S, H, D] from per-core out^T [HPC, D, S].

Schedule (v2): q blocks in order [1..NQ-1, 0] so the first block only needs
the first K^T segment (DMA staged, compute starts ~1.5us in, not ~20us).
The previous block's PV chunk matmuls are drained between QK score groups so
the ACT engine (the bottleneck at ~138us busy) never starves behind a long
PV burst. The final block runs its own PV eagerly to cut the tail.
"""
import math
import os
import sys
from collections import deque

for _p in ("/opt/trn_rl_repo", "/root/.axon_site/_ro/trn_rl_repo"):
    if os.path.isdir(_p) and _p not in sys.path:
        sys.path.insert(0, _p)

import numpy as np

import concourse.bass as bass  # noqa: E402
import concourse.mybir as mybir  # noqa: E402
import concourse.tile as tile  # noqa: E402
from concourse import bacc  # noqa: E402
from concourse.bass_utils import run_bass_kernel_spmd  # noqa: E402
from concourse.masks import make_upper_triangular  # noqa: E402

N_CORES = 8
CH = 128  # key/t chunk (PE contraction width)
QB = 512  # query block (PE moving width / PSUM bank)
GRP = 3   # score chunks per ACT group (3 PSUM banks)

F16 = mybir.dt.float16
F32 = mybir.dt.float32


def build(S=4096, HPC=2, qk_dt=F16, pv_dt=F16, reps=1):
    """Build + compile the per-core Bass program (identical on all cores)."""
    NQ = S // QB
    NCH = S // CH
    RPB = QB // CH  # chunks per q-block row of the diagonal (4)
    np_qk = mybir.dt.np(qk_dt)
    np_pv = mybir.dt.np(pv_dt)

    build.grp_ctr = 0
    nc = bacc.Bacc("TRN2", target_bir_lowering=False, debug=False,
                   num_devices=N_CORES)
    qT_d = nc.declare_dram_parameter("qT", [HPC, 128, S], qk_dt, isOutput=False)
    kT_d = nc.declare_dram_parameter("kT", [HPC, 128, S], qk_dt, isOutput=False)
    v_d = nc.declare_dram_parameter("v", [HPC, S, 128], pv_dt, isOutput=False)
    o_d = nc.declare_dram_parameter("outT", [HPC, 128, S], F32, isOutput=True)

    scale = 1.0 / math.sqrt(128.0)

    KSPLIT = 4
    KSEG = S // KSPLIT
    NSEG = NCH // KSPLIT

    with tile.TileContext(nc) as tc:
        with (
            tc.tile_pool(name="const", bufs=1) as constp,
            tc.tile_pool(name="kv", bufs=1) as kvp,
            tc.tile_pool(name="qs", bufs=4) as qsp,
            tc.tile_pool(name="panel", bufs=3) as panelp,
            tc.tile_pool(name="red", bufs=6) as redp,
            tc.tile_pool(name="tb", bufs=3) as tbp,
            tc.tile_pool(name="aux", bufs=3) as auxp,
            tc.tile_pool(name="outp", bufs=4) as outpp,
            tc.tile_pool(name="ps_sc", bufs=2, space="PSUM") as ps_sc,
            tc.tile_pool(name="ps_pv", bufs=1, space="PSUM") as ps_pv,
            tc.tile_pool(name="ps_l", bufs=1, space="PSUM") as ps_l,
        ):
            ones_sb = constp.tile([128, 128], pv_dt, tag="ones")
            nc.gpsimd.memset(ones_sb[:], 1.0)
            tri = constp.tile([128, 128], pv_dt, tag="tri")
            make_upper_triangular(nc, tri[:], val=1.0, diag=True)

            kT_sb = [[None] * KSPLIT for _ in range(HPC)]
            v_sb = [[None] * KSPLIT for _ in range(HPC)]

            def load_kT(h, s_):
                kt = kvp.tile([128, KSEG], qk_dt, tag=f"kT{h}_{s_}")
                nc.sync.dma_start(
                    kt[:], kT_d.ap()[h][:, s_ * KSEG:(s_ + 1) * KSEG])
                kT_sb[h][s_] = kt

            def load_v(h, s_):
                vt = kvp.tile([128, NSEG, 128], pv_dt, tag=f"v{h}_{s_}")
                nc.sync.dma_start(
                    vt[:],
                    v_d.ap()[h][s_ * NSEG * 128:(s_ + 1) * NSEG * 128, :]
                    .rearrange("(c p) d -> p c d", p=128))
                v_sb[h][s_] = vt

            def kT_chunk(h, c):
                t0 = c * CH
                return kT_sb[h][t0 // KSEG][:, t0 % KSEG:t0 % KSEG + CH]

            def v_chunk(h, c):
                return v_sb[h][c // NSEG][:, c % NSEG, :]

            def emit_pv_chunk(st, c):
                # one PV accumulation member: pvp += V[c].T-free @ P^T[c]
                h, C = st["h"], st["C"]
                if st.get("pvp") is None:
                    pvp = ps_pv.tile([128, QB], F32, tag="pv")
                    st["pvp"] = pvp
                k = max(0, c - (C - RPB))
                q0 = k * CH
                nc.tensor.matmul(
                    st["pvp"][:, q0:QB], v_chunk(h, c),
                    st["pan"][:, c * QB + q0:(c + 1) * QB],
                    start=(c == 0), stop=(c == C - 1),
                    skip_group_check=True)

            def emit_l(st):
                lp = ps_l.tile([128, QB], F32, tag="l")
                nc.tensor.matmul(lp[:], ones_sb[:], st["tb"][:, :QB],
                                 start=True, stop=True)
                st["lp"] = lp

            def emit_finish(st, fast=False):
                h, qi = st["h"], st["qi"]
                linv = auxp.tile([128, QB], F32, tag="linv")
                nc.vector.reciprocal(linv[:], st["lp"][:])
                ot = outpp.tile([128, QB], F32, tag="ot")
                nc.vector.tensor_mul(ot[:], st["pvp"][:], linv[:])
                nc.sync.dma_start(o_d.ap()[h][:, qi * QB:(qi + 1) * QB], ot[:])

            order = list(range(2, NQ)) + [0, 1]
            blocks = [(h, qi) for h in range(HPC) for qi in order]

            prev = None
            pending = deque()  # PV chunk indices of `prev` not yet emitted
            for _rep in range(reps):
              for bi, (h, qi) in enumerate(blocks):
                last = (_rep == reps - 1) and (bi == len(blocks) - 1)
                C = (qi + 1) * RPB  # causal chunk count for this q block
                qsl = qsp.tile([128, QB], qk_dt, tag="qsl")
                nc.sync.dma_start(
                    qsl[:], qT_d.ap()[h][:, qi * QB:(qi + 1) * QB])
                if qi == order[0] and _rep == 0:
                    # stage this head's K/V loads (after the q-slab DMA so the
                    # first QK isn't queued behind them); K segs before the V
                    # segs they compute against
                    if h == 0:
                        for ss, what in ((0, "k"), (1, "k"), (0, "v"),
                                         (2, "k"), (3, "k"), (1, "v"),
                                         (2, "v"), (3, "v")):
                            (load_kT if what == "k" else load_v)(h, ss)
                    else:
                        for ss in range(KSPLIT):
                            load_kT(h, ss)
                        for ss in range(KSPLIT):
                            load_v(h, ss)
                pan = panelp.tile([128, NCH * QB], pv_dt, tag="panel")
                main = C - 3
                n_groups = (main + GRP - 1) // GRP + 1  # + diag suffix
                drain = ((len(pending) + n_groups - 1) // n_groups
                         if pending else 0)
                cur = dict(h=h, qi=qi, C=C, pan=pan, acc=None, red_pos=0)
                eager_next = 0
                CLEAN = (C - RPB) * QB  # cols before the diag region

                def feed(limit, st=cur):
                    # streaming row-sum: fold pan columns into a running
                    # [128, 2*QB] accumulator, pair-add first (tree-optimal
                    # cost), spread across the block instead of an
                    # end-of-block burst that clogs the DVE FIFO
                    while limit - st["red_pos"] >= 4 * QB:
                        rp = st["red_pos"]
                        if st["acc"] is None:
                            acc = redp.tile([128, 2 * QB], pv_dt, tag="acc")
                            nc.vector.tensor_add(
                                acc[:], st["pan"][:, rp:rp + 2 * QB],
                                st["pan"][:, rp + 2 * QB:rp + 4 * QB])
                            st["acc"] = acc
                        else:
                            t = redp.tile([128, 2 * QB], pv_dt, tag="rt")
                            nc.vector.tensor_add(
                                t[:], st["pan"][:, rp:rp + 2 * QB],
                                st["pan"][:, rp + 2 * QB:rp + 4 * QB])
                            nc.vector.tensor_add(
                                st["acc"][:], st["acc"][:], t[:])
                        st["red_pos"] = rp + 4 * QB

                # QK chunks -> PSUM groups -> exp -> panel, PV(prev) drained
                # between groups to keep ACT fed and PE dense. The final
                # block instead runs its own PV eagerly, one group behind
                # the exp, so the tail after the last exp stays short.
                for g0 in range(0, main, GRP):
                    n = min(GRP, main - g0)
                    sc = ps_sc.tile([128, GRP * QB], F32, tag="sc")
                    for j in range(n):
                        c = g0 + j
                        nc.tensor.matmul(
                            sc[:, j * QB:(j + 1) * QB],
                            kT_chunk(h, c), qsl[:],
                            start=True, stop=True)
                    offload = (n == GRP and not last
                               and bi not in (0, len(blocks) - 2)
                               and build.grp_ctr % 11 == 5)
                    if n == GRP:
                        build.grp_ctr += 1
                    if offload:
                        # Schraudolph exp on DVE: fp16 bit code =
                        # s*(1024*log2e*scale) + (15360 - 45), written as
                        # int16 into the fp16 panel (ACT is the bottleneck;
                        # this moves ~1/6 of the exp work to DVE)
                        nc.vector.tensor_scalar(
                            pan[:, g0 * QB:(g0 + n) * QB]
                            .bitcast(mybir.dt.int16),
                            sc[:, :n * QB],
                            1024.0 * 1.4426950408889634 * scale, 15315.0,
                            mybir.AluOpType.mult, mybir.AluOpType.add)
                    else:
                        nc.scalar.activation(
                            pan[:, g0 * QB:(g0 + n) * QB], sc[:, :n * QB],
                            mybir.ActivationFunctionType.Exp, scale=scale)
                    feed(min((g0 + n) * QB, CLEAN))
                    if last and g0 == 0:
                        # retire prev completely so the pv bank is free
                        # before this block's own eager PV needs it
                        while pending:
                            emit_pv_chunk(prev, pending.popleft())
                        if prev is not None:
                            emit_l(prev)
                            emit_finish(prev, fast=True)
                    elif last:
                        # eager PV for chunks of groups already exp'd (safe:
                        # strictly below the diagonal, no fix-up needed);
                        # hold back c = C-4 (the k=0 diag chunk)
                        while eager_next < min(g0, main - 1):
                            emit_pv_chunk(cur, eager_next)
                            eager_next += 1
                    else:
                        for _ in range(min(drain, len(pending))):
                            emit_pv_chunk(prev, pending.popleft())
                # 3-chunk diag suffix: skips the fully-masked q < 128 prefix
                sc = ps_sc.tile([128, GRP * QB], F32, tag="sc")
                for j in range(3):
                    c = C - 3 + j
                    nc.tensor.matmul(
                        sc[:, j * QB + CH:(j + 1) * QB],
                        kT_chunk(h, c), qsl[:, CH:QB],
                        start=True, stop=True)
                nc.scalar.activation(
                    pan[:, (C - 3) * QB:C * QB].rearrange(
                        "p (j q) -> p j q", q=QB)[:, :, CH:],
                    sc[:].rearrange("p (j q) -> p j q", q=QB)[:, :, CH:],
                    mybir.ActivationFunctionType.Exp, scale=scale)
                # causal fix-up on the RPB diagonal chunks
                for k in range(RPB):
                    off = (C - RPB + k) * QB
                    if k > 0:
                        nc.gpsimd.memset(pan[:, off:off + k * CH], 0.0)
                    sl = pan[:, off + k * CH:off + (k + 1) * CH]
                    nc.vector.tensor_mul(sl, sl, tri[:])
                if not last:
                    # flush remaining PV(prev) + close it out
                    while pending:
                        emit_pv_chunk(prev, pending.popleft())
                    if prev is not None:
                        emit_l(prev)
                        emit_finish(prev, fast=(bi == len(blocks) - 1))
                # remaining row-sum: diag region (post-fixup) + fold to tb
                feed(C * QB)
                tb = tbp.tile([128, QB], pv_dt, tag="tb")
                nc.vector.tensor_add(
                    tb[:], cur["acc"][:, :QB], cur["acc"][:, QB:2 * QB])
                cur["tb"] = tb
                prev = cur
                if last:
                    # remaining eager PV: held-back main chunks + diag chunks
                    while eager_next < C:
                        emit_pv_chunk(cur, eager_next)
                        eager_next += 1
                    emit_l(cur)
                    emit_finish(cur, fast=True)
                    prev = None
                else:
                    pending = deque(range(C))

    nc.compile()
    return nc, np_qk, np_pv


_CACHE = {}


def _get(S, HPC):
    key = (S, HPC)
    if key not in _CACHE:
        _CACHE[key] = build(S, HPC)
    return _CACHE[key]


def kernel(query, key, value):
    q = np.asarray(query)
    k = np.asarray(key)
    v = np.asarray(value)
    B, S, H, D = q.shape
    assert B == 1 and D == 128 and H % N_CORES == 0
    HPC = H // N_CORES
    nc, np_qk, np_pv = _get(S, HPC)

    in_maps = []
    for c in range(N_CORES):
        hh = slice(c * HPC, (c + 1) * HPC)
        qT = np.ascontiguousarray(
            q[0, :, hh, :].astype(np_qk).transpose(1, 2, 0))
        kT = np.ascontiguousarray(
            k[0, :, hh, :].astype(np_qk).transpose(1, 2, 0))
        vv = np.ascontiguousarray(
            v[0, :, hh, :].astype(np_pv).transpose(1, 0, 2))
        in_maps.append({"qT": qT, "kT": kT, "v": vv})

    res = run_bass_kernel_spmd(nc, in_maps, list(range(N_CORES)))

    out = np.empty((B, S, H, D), np.float32)
    for c in range(N_CORES):
        oT = res.results[c]["outT"]  # [HPC, 128, S] fp32
        out[0, :, c * HPC:(c + 1) * HPC, :] = oT.transpose(2, 0, 1)
    return out.astype(query.dtype)
